# revision 11
# baseline (speedup 1.0000x reference)
"""Trainium2 Bass kernel for the ImaginationCore rollout.

Strategy
--------
The reference unrolls T=17 transition steps over B=2048 rows, but a row only
updates while it is "active", and the active set shrinks monotonically: a row
that is done (player==0 or winner>0) or whose player equals the initial player
freezes forever (its state stops changing, so its gate never changes).  For the
benchmark inputs the active counts per step are [449, 14, 0, 0, ...], so the
real work is two small ragged steps, not 17 dense ones.

The kernel runs host-side control flow and ships the dense GEMMs (>99.9% of
the FLOPs) to the 8 NeuronCores:

  * z1 = s @ W1 (active rows): contraction dim S=12288 sharded over the 8
    cores; partial products summed on host.  Runs as a bfloat16 hi/lo 3-pass
    split (stacked along K: out = x_hi'W_hi + x_lo'W_hi + x_hi'W_lo) with fp32
    PSUM accumulation -> ~1e-5 absolute error, so every downstream argmax
    decision matches the fp32 reference, at full bf16 PE rate.
  * logits = s @ Wa (steps >= 1): single-pass fp8e4 (Wa pre-scaled by 64 so
    its entries are normal in e4m3; scale is irrelevant for candidate ranking),
    K-sharded, fused into the same NEFF as z1.  The host recomputes the top-32
    candidate columns per row exactly in fp32 and argmaxes over those, so the
    chosen action is exactly the fp32 argmax (typical top-2 logit gaps are
    ~0.1, far above the ~2e-2 fp8 noise; the true argmax falling outside the
    fp8 top-32 would need ~32 spurious outrankers, probability ~0).
  * z2 = h @ W2: output dim S sharded over the 8 cores, single-pass fp16
    (~2e-4 max error on the bulk of the state after the sigmoid).  Exactness is
    restored where it matters on host from the exact fp32 h:
      - the 12 decision/reward columns (player/winner one-hots, DMG/ACC/VLN)
        are recomputed exactly, so done/active gates and rewards are exact;
      - rows that remain active into the next step are recomputed exactly in
        full before being fed back into the next step's GEMMs.
    Only frozen rows keep the ~2e-4 bulk-state error in the final output.

Everything else - masks, argmax, gathers, tanh/sigmoid, reward accumulation -
is O(B) or O(n*H) host work.
"""

import time

import numpy as np
import ml_dtypes

import concourse.bacc as bacc
import concourse.mybir as mybir
import concourse.tile as tile
from concourse.kernels.tile_matmul import matmul_tile_kernel
from concourse.bass_utils import run_bass_kernel_spmd

# ---- problem dimensions (hardcoded per spec) ----
P = 128
B = 2048
S = 12288
H = 1024
NA = 2312
T = 17
NCORES = 8
KS = S // NCORES          # 1536: K-shard size of S
NA_PAD = 2560             # Wa cols zero-padded so each core's share is 320
WA_SCALE = 64.0           # fp8 scale for Wa (its entries are subnormal in e4m3)
NW2 = S // NCORES         # 1536 cols of W2 per core

# state layout indices (mirror reference.py)
IDX_MY_DMG, IDX_MY_ACC, IDX_MY_VLN = 110, 111, 112
IDX_EN_DMG, IDX_EN_ACC, IDX_EN_VLN = 160, 161, 162
# columns that drive gates and rewards; rescued exactly on host
COLS_EXACT = np.array([0, 1, 2, 3, 4, 5,
                       IDX_MY_DMG, IDX_MY_ACC, IDX_MY_VLN,
                       IDX_EN_DMG, IDX_EN_ACC, IDX_EN_VLN])

BF16 = ml_dtypes.bfloat16
FP8 = ml_dtypes.float8_e4m3
BF = mybir.dt.bfloat16

# Optional profiling: when PROFILE is True, each distinct module is timed once
# with the concourse cost-model timeline simulator; per-launch times accumulate
# into HW_NS.
PROFILE = False
HW_NS = 0.0
_SIM_NS_CACHE = {}

_MOD_CACHE = {}


def _build_module(specs):
    """One NEFF running len(specs) independent GEMMs.

    specs: tuple of (K, M, N, dt); each GEMM computes
    out[M, N] = kxm[K, M].T @ kxn[K, N] with fp32 output.
    """
    if specs in _MOD_CACHE:
        return _MOD_CACHE[specs]
    nc = bacc.Bacc(None, target_bir_lowering=False)
    names = []
    handles = []
    with tile.TileContext(nc) as tc:
        with tc.tile_pool(name="dram", bufs=1, space="DRAM") as dram:
            for (K, M, N, dt) in specs:
                kxm = dram.tile((P, K // P, M), dt, kind="ExternalInput")
                kxn = dram.tile((P, K // P, N), dt, kind="ExternalInput")
                mxn = dram.tile((P, M // P, N), mybir.dt.float32,
                                kind="ExternalOutput")
                names.append((kxm.name, kxn.name, mxn.name))
                handles.append((kxm, kxn, mxn))
            for (kxm, kxn, mxn) in handles:
                matmul_tile_kernel(tc, kxm[:], kxn[:], mxn[:])
    nc.compile()
    _MOD_CACHE[specs] = (nc, names)
    return nc, names


def _to3d(x):
    """[K, M] -> (128, K//128, M) with partition = K % 128 innermost."""
    K, M = x.shape
    return np.ascontiguousarray(x.reshape(K // P, P, M).transpose(1, 0, 2))


def _from2d(x3):
    """(128, M//128, N) -> [M, N]."""
    p, mo, n = x3.shape
    return x3.transpose(1, 0, 2).reshape(mo * p, n)


def _run_module(specs, per_core_inputs):
    """per_core_inputs[g][i] = (kxm, kxn) for GEMM g on core i.
    Returns outs[g][i] = [M, N] fp32."""
    global HW_NS
    nc, names = _build_module(specs)
    in_maps = []
    for i in range(NCORES):
        m = {}
        for g, (a, b, _c) in enumerate(names):
            kxm, kxn = per_core_inputs[g][i]
            m[a] = _to3d(kxm)
            m[b] = _to3d(kxn)
        in_maps.append(m)
    last_err = None
    for _attempt in range(3):
        try:
            res = run_bass_kernel_spmd(nc, in_maps, core_ids=list(range(NCORES)))
            break
        except Exception as e:  # transient backend flakes: retry
            last_err = e
            time.sleep(2.0)
    else:
        raise last_err
    if PROFILE:
        if specs not in _SIM_NS_CACHE:
            from concourse.timeline_sim import TimelineSim
            _SIM_NS_CACHE[specs] = TimelineSim(nc).simulate()
        HW_NS += _SIM_NS_CACHE[specs]
    return [[_from2d(res.results[i][c]) for i in range(NCORES)]
            for (_a, _b, c) in names]


def _split_hi_lo(x):
    hi = x.astype(BF16)
    lo = (x - hi.astype(np.float32)).astype(BF16)
    return hi, lo


def _stack_x(x):
    """kxm-side 3-pass stack: [x_hi; x_lo; x_hi] along K."""
    hi, lo = _split_hi_lo(x)
    return np.concatenate([hi, lo, hi], axis=0)


def _stack_w(w):
    """kxn-side 3-pass stack: [w_hi; w_hi; w_lo] along K."""
    hi, lo = _split_hi_lo(w)
    return np.concatenate([hi, hi, lo], axis=0)


def _pad_rows_T(x, n_pad, dtype=np.float32):
    """[n, D] -> transposed + column-padded [D, n_pad]."""
    n, d = x.shape
    out = np.zeros((d, n_pad), dtype)
    out[:, :n] = x.T.astype(dtype)
    return out


def _sigmoid(x):
    out = np.empty_like(x, np.float32)
    pos = x >= 0
    out[pos] = 1.0 / (1.0 + np.exp(-x[pos]))
    ex = np.exp(x[~pos])
    out[~pos] = ex / (1.0 + ex)
    return out


def _npad(n):
    return max(P, ((n + P - 1) // P) * P)


class _Weights:
    """Per-call cache of host-side weight layouts."""

    def __init__(self, W1, b1, W2, act_emb, Wa):
        self.W1 = np.asarray(W1, np.float32)
        self.b1 = np.asarray(b1, np.float32)
        self.W2 = np.asarray(W2, np.float32)
        self.act_emb = np.asarray(act_emb, np.float32)
        self.Wa = np.asarray(Wa, np.float32)
        self._w1_shards = None
        self._w2_shards = None
        self._wa_shards = None

    def w1_shards(self):  # 3-pass stacked K-shards of W1
        if self._w1_shards is None:
            self._w1_shards = [
                _stack_w(self.W1[c * KS:(c + 1) * KS, :]) for c in range(NCORES)
            ]
        return self._w1_shards

    def w2_shards(self):  # single-pass fp16 column shards of W2
        if self._w2_shards is None:
            self._w2_shards = [
                np.ascontiguousarray(
                    self.W2[:, c * NW2:(c + 1) * NW2].astype(np.float16))
                for c in range(NCORES)
            ]
        return self._w2_shards

    def wa_shards(self):  # single-pass fp8 K-shards of Wa (cols padded, scaled)
        if self._wa_shards is None:
            wa_pad = np.zeros((S, NA_PAD), FP8)
            wa_pad[:, :NA] = (self.Wa * np.float32(WA_SCALE)).astype(FP8)
            self._wa_shards = [
                np.ascontiguousarray(wa_pad[c * KS:(c + 1) * KS, :])
                for c in range(NCORES)
            ]
        return self._wa_shards


def _z1_spec(n_pad):
    return (3 * KS, n_pad, H, BF)


def _logits_spec(n_pad):
    return (KS, n_pad, NA_PAD, mybir.dt.float8e4)


def _w2_spec(n_pad):
    return (H, n_pad, NW2, mybir.dt.float16)


def _z1_inputs(sT, wts):
    """GEMM inputs for z1 = s @ W1, K-sharded, 3-pass."""
    return [(_stack_x(sT[c * KS:(c + 1) * KS, :]), wts.w1_shards()[c])
            for c in range(NCORES)]


def _logits_inputs(sT_f8, wts):
    """GEMM inputs for logits = s @ Wa, K-sharded, single-pass fp8."""
    return [(np.ascontiguousarray(sT_f8[c * KS:(c + 1) * KS, :]),
             wts.wa_shards()[c]) for c in range(NCORES)]


def _exact_actions(logits_bf, s_rows, wts):
    """Exact fp32 argmax via host recompute of the bf16 top-32 candidates."""
    n = s_rows.shape[0]
    ncand = min(32, NA)
    cand = np.argpartition(-logits_bf, ncand - 1, axis=1)[:, :ncand]
    cand.sort(axis=1)  # ties in exact fp32 break to the lowest column index
    actions = np.empty(n, np.int64)
    for i in range(n):
        exact = s_rows[i] @ wts.Wa[:, cand[i]]
        actions[i] = cand[i][np.argmax(exact)]
    return actions


def _w2_apply(h, n_pad, wts):
    """Approximate ns = sigmoid(h @ W2) (single-pass bf16) + exact h."""
    hT_bf = _pad_rows_T(h, n_pad, np.float16)
    inputs = [(hT_bf, wts.w2_shards()[c]) for c in range(NCORES)]
    (outs,) = _run_module((_w2_spec(n_pad),), [inputs])
    n = h.shape[0]
    z2 = np.concatenate(outs, axis=1)[:n]
    return _sigmoid(z2)


def _step_reward(s):
    return np.float32(1000.0) * (
        s[:, IDX_EN_VLN] - s[:, IDX_MY_VLN]
        + (s[:, IDX_EN_DMG] - s[:, IDX_MY_DMG]))


def _players_winners(s):
    return np.argmax(s[:, 0:3], axis=1), np.argmax(s[:, 3:6], axis=1)


def kernel(initial_state, initial_action, W1, b1, W2, act_emb, Wa):
    global HW_NS
    HW_NS = 0.0
    state = np.array(initial_state, np.float32, copy=True)
    action0 = np.asarray(initial_action).astype(np.int64)
    wts = _Weights(W1, b1, W2, act_emb, Wa)

    player, winner = _players_winners(state)
    initial_player = player.copy()
    initial_done = (player == 0) | (winner > 0)
    reward = np.zeros(B, np.float32)

    # rows updated by the previous step, with their exact h (for rescue)
    prev_idx = None
    prev_h = None

    for i in range(T):
        player, winner = _players_winners(state)
        done = (player == 0) | (winner > 0)
        if i == 0:
            active = ~done
        else:
            active = (~done) & (player != initial_player)
        idx = np.where(active)[0]
        if idx.size == 0:
            break  # state is frozen for this and every later step

        if prev_idx is not None:
            # Rows entering this step's GEMMs were updated last step with a
            # single-pass-bf16 W2 product; recompute them exactly on host.
            need = np.intersect1d(idx, prev_idx)
            if need.size:
                pos = np.searchsorted(prev_idx, need)
                state[need] = _sigmoid(prev_h[pos] @ wts.W2)

        s_rows = state[idx]
        n = idx.size
        n_pad = _npad(n)
        sT = _pad_rows_T(s_rows, n_pad)

        if i == 0:
            act_rows = action0[idx]
            (z1_outs,) = _run_module(
                (_z1_spec(n_pad),), [_z1_inputs(sT, wts)])
            logits_bf = None
        else:
            # fused NEFF: z1 (3-pass) + action logits (single-pass bf16)
            z1_outs, lg_outs = _run_module(
                (_z1_spec(n_pad), _logits_spec(n_pad)),
                [_z1_inputs(sT, wts),
                 _logits_inputs(sT.astype(FP8), wts)])
            logits_bf = np.zeros((n_pad, NA_PAD), np.float32)
            for o in lg_outs:
                logits_bf += o
            act_rows = _exact_actions(logits_bf[:n, :NA], s_rows, wts)

        z1 = z1_outs[0]
        for o in z1_outs[1:]:
            z1 = z1 + o
        h = np.tanh(z1[:n] + wts.act_emb[act_rows] + wts.b1).astype(np.float32)

        ns = _w2_apply(h, n_pad, wts)
        state[idx] = ns
        # exact rescue of the decision + reward columns
        zc = h @ wts.W2[:, COLS_EXACT]
        state[np.ix_(idx, COLS_EXACT)] = _sigmoid(zc)
        reward[idx] = (reward[idx] + _step_reward(state)[idx]).astype(np.float32)
        prev_idx, prev_h = idx, h

    player, winner = _players_winners(state)
    done = (player == 0) | (winner > 0)
    reward = (reward + np.where(~initial_done, np.float32(-0.01),
                                np.float32(0.0))).astype(np.float32)
    term = done & ~initial_done
    term_r = _step_reward(state) + np.float32(1000.0) * (
        state[:, IDX_EN_ACC] - state[:, IDX_MY_ACC])
    reward = (reward + np.where(term, term_r, np.float32(0.0))).astype(np.float32)
    return state, reward, done


# revision 15
# speedup vs baseline: 1.2156x; 1.2156x over previous
"""Trainium2 Bass kernel for the ImaginationCore rollout.

Strategy
--------
The reference unrolls T=17 transition steps over B=2048 rows, but a row only
updates while it is "active", and the active set shrinks monotonically: a row
that is done (player==0 or winner>0) or whose player equals the initial player
freezes forever (its state stops changing, so its gate never changes).  For the
benchmark inputs the active counts per step are [449, 14, 0, 0, ...], so the
real work is two small ragged steps, not 17 dense ones.

The kernel runs host-side control flow and ships the dense GEMMs (>99.9% of
the FLOPs) to the 8 NeuronCores:

  * z1 = s @ W1 (active rows): contraction dim S=12288 sharded over the 8
    cores; partial products summed on host.  Runs as a bfloat16 hi/lo 3-pass
    split (stacked along K: out = x_hi'W_hi + x_lo'W_hi + x_hi'W_lo) with fp32
    PSUM accumulation -> ~1e-5 absolute error, so every downstream argmax
    decision matches the fp32 reference, at full bf16 PE rate.
  * logits = s @ Wa (steps >= 1): single-pass fp8e4 (Wa pre-scaled by 64 so
    its entries are normal in e4m3; scale is irrelevant for candidate ranking),
    K-sharded, fused into the same NEFF as z1.  The host recomputes the top-32
    candidate columns per row exactly in fp32 and argmaxes over those, so the
    chosen action is exactly the fp32 argmax (typical top-2 logit gaps are
    ~0.1, far above the ~2e-2 fp8 noise; the true argmax falling outside the
    fp8 top-32 would need ~32 spurious outrankers, probability ~0).
  * z2 = h @ W2: output dim S sharded over the 8 cores, single-pass fp16
    (~2e-4 max error on the bulk of the state after the sigmoid).  Exactness is
    restored where it matters on host from the exact fp32 h:
      - the 12 decision/reward columns (player/winner one-hots, DMG/ACC/VLN)
        are recomputed exactly, so done/active gates and rewards are exact;
      - rows that remain active into the next step are recomputed exactly in
        full before being fed back into the next step's GEMMs.
    Only frozen rows keep the ~2e-4 bulk-state error in the final output.

Everything else - masks, argmax, gathers, tanh/sigmoid, reward accumulation -
is O(B) or O(n*H) host work.
"""

import time

import numpy as np
import ml_dtypes

import concourse.bacc as bacc
import concourse.mybir as mybir
import concourse.tile as tile
from concourse.kernels.tile_matmul import matmul_tile_kernel
from concourse.bass_utils import run_bass_kernel_spmd

# ---- problem dimensions (hardcoded per spec) ----
P = 128
B = 2048
S = 12288
H = 1024
NA = 2312
T = 17
NCORES = 8
KS = S // NCORES          # 1536: K-shard size of S
NA_PAD = 2560             # Wa cols zero-padded so each core's share is 320
WA_SCALE = 64.0           # fp8 scale for Wa (its entries are subnormal in e4m3)
NW2 = S // NCORES         # 1536 cols of W2 per core

# state layout indices (mirror reference.py)
IDX_MY_DMG, IDX_MY_ACC, IDX_MY_VLN = 110, 111, 112
IDX_EN_DMG, IDX_EN_ACC, IDX_EN_VLN = 160, 161, 162
# columns that drive gates and rewards; rescued exactly on host
COLS_EXACT = np.array([0, 1, 2, 3, 4, 5,
                       IDX_MY_DMG, IDX_MY_ACC, IDX_MY_VLN,
                       IDX_EN_DMG, IDX_EN_ACC, IDX_EN_VLN])

BF16 = ml_dtypes.bfloat16
FP8 = ml_dtypes.float8_e4m3
BF = mybir.dt.bfloat16

# Optional profiling: when PROFILE is True, each distinct module is timed once
# with the concourse cost-model timeline simulator; per-launch times accumulate
# into HW_NS.
PROFILE = False
HW_NS = 0.0
_SIM_NS_CACHE = {}

_MOD_CACHE = {}


def _build_module(specs):
    """One NEFF running len(specs) independent GEMMs.

    specs: tuple of (K, M, N, dt); each GEMM computes
    out[M, N] = kxm[K, M].T @ kxn[K, N] with fp32 output.
    """
    if specs in _MOD_CACHE:
        return _MOD_CACHE[specs]
    nc = bacc.Bacc(None, target_bir_lowering=False)
    names = []
    handles = []
    with tile.TileContext(nc) as tc:
        with tc.tile_pool(name="dram", bufs=1, space="DRAM") as dram:
            for (K, M, N, dt) in specs:
                kxm = dram.tile((P, K // P, M), dt, kind="ExternalInput")
                kxn = dram.tile((P, K // P, N), dt, kind="ExternalInput")
                mxn = dram.tile((P, M // P, N), mybir.dt.float32,
                                kind="ExternalOutput")
                names.append((kxm.name, kxn.name, mxn.name))
                handles.append((kxm, kxn, mxn))
            # Issue the first spec (the heavy 3-pass z1 GEMM) last: measured
            # (TimelineSim, all 6 orders) to give the best Tile schedule —
            # the light GEMMs' DMAs overlap the big GEMM's compute.
            order = list(range(1, len(specs))) + [0]
            for g in order:
                kxm, kxn, mxn = handles[g]
                matmul_tile_kernel(tc, kxm[:], kxn[:], mxn[:])
    nc.compile()
    _MOD_CACHE[specs] = (nc, names)
    return nc, names


def _to3d(x):
    """[K, M] -> (128, K//128, M) with partition = K % 128 innermost."""
    K, M = x.shape
    return np.ascontiguousarray(x.reshape(K // P, P, M).transpose(1, 0, 2))


def _from2d(x3):
    """(128, M//128, N) -> [M, N]."""
    p, mo, n = x3.shape
    return x3.transpose(1, 0, 2).reshape(mo * p, n)


def _run_module(specs, per_core_inputs):
    """per_core_inputs[g][i] = (kxm, kxn) for GEMM g on core i.
    Returns outs[g][i] = [M, N] fp32."""
    global HW_NS
    nc, names = _build_module(specs)
    in_maps = []
    for i in range(NCORES):
        m = {}
        for g, (a, b, _c) in enumerate(names):
            kxm, kxn = per_core_inputs[g][i]
            m[a] = _to3d(kxm)
            m[b] = _to3d(kxn)
        in_maps.append(m)
    last_err = None
    for _attempt in range(3):
        try:
            res = run_bass_kernel_spmd(nc, in_maps, core_ids=list(range(NCORES)))
            break
        except Exception as e:  # transient backend flakes: retry
            last_err = e
            time.sleep(2.0)
    else:
        raise last_err
    if PROFILE:
        if specs not in _SIM_NS_CACHE:
            from concourse.timeline_sim import TimelineSim
            _SIM_NS_CACHE[specs] = TimelineSim(nc).simulate()
        HW_NS += _SIM_NS_CACHE[specs]
    return [[_from2d(res.results[i][c]) for i in range(NCORES)]
            for (_a, _b, c) in names]


def _split_hi_lo(x):
    hi = x.astype(BF16)
    lo = (x - hi.astype(np.float32)).astype(BF16)
    return hi, lo


def _stack_x(x):
    """kxm-side 3-pass stack: [x_hi; x_lo; x_hi] along K."""
    hi, lo = _split_hi_lo(x)
    return np.concatenate([hi, lo, hi], axis=0)


def _stack_w(w):
    """kxn-side 3-pass stack: [w_hi; w_hi; w_lo] along K."""
    hi, lo = _split_hi_lo(w)
    return np.concatenate([hi, hi, lo], axis=0)


def _pad_rows_T(x, n_pad, dtype=np.float32):
    """[n, D] -> transposed + column-padded [D, n_pad]."""
    n, d = x.shape
    out = np.zeros((d, n_pad), dtype)
    out[:, :n] = x.T.astype(dtype)
    return out


def _sigmoid(x):
    out = np.empty_like(x, np.float32)
    pos = x >= 0
    out[pos] = 1.0 / (1.0 + np.exp(-x[pos]))
    ex = np.exp(x[~pos])
    out[~pos] = ex / (1.0 + ex)
    return out


def _npad(n):
    return max(P, ((n + P - 1) // P) * P)


class _Weights:
    """Per-call cache of host-side weight layouts."""

    def __init__(self, W1, b1, W2, act_emb, Wa):
        self.W1 = np.asarray(W1, np.float32)
        self.b1 = np.asarray(b1, np.float32)
        self.W2 = np.asarray(W2, np.float32)
        self.act_emb = np.asarray(act_emb, np.float32)
        self.Wa = np.asarray(Wa, np.float32)
        self._w1_shards = None
        self._w2_shards = None
        self._wa_shards = None

    def w1_shards(self):  # 3-pass stacked K-shards of W1
        if self._w1_shards is None:
            self._w1_shards = [
                _stack_w(self.W1[c * KS:(c + 1) * KS, :]) for c in range(NCORES)
            ]
        return self._w1_shards

    def w2_shards(self):  # single-pass fp16 column shards of W2
        if self._w2_shards is None:
            self._w2_shards = [
                np.ascontiguousarray(
                    self.W2[:, c * NW2:(c + 1) * NW2].astype(np.float16))
                for c in range(NCORES)
            ]
        return self._w2_shards

    def wa_shards(self):  # single-pass fp8 K-shards of Wa (cols padded, scaled)
        if self._wa_shards is None:
            wa_pad = np.zeros((S, NA_PAD), FP8)
            wa_pad[:, :NA] = (self.Wa * np.float32(WA_SCALE)).astype(FP8)
            self._wa_shards = [
                np.ascontiguousarray(wa_pad[c * KS:(c + 1) * KS, :])
                for c in range(NCORES)
            ]
        return self._wa_shards


def _z1_spec(n_pad):
    return (3 * KS, n_pad, H, BF)


def _logits_spec(n_pad):
    return (KS, n_pad, NA_PAD, mybir.dt.float8e4)


def _w2_spec(n_pad):
    return (H, n_pad, NW2, mybir.dt.float16)


def _z1_inputs(sT, wts):
    """GEMM inputs for z1 = s @ W1, K-sharded, 3-pass."""
    return [(_stack_x(sT[c * KS:(c + 1) * KS, :]), wts.w1_shards()[c])
            for c in range(NCORES)]


def _logits_inputs(sT_f8, wts):
    """GEMM inputs for logits = s @ Wa, K-sharded, single-pass fp8."""
    return [(np.ascontiguousarray(sT_f8[c * KS:(c + 1) * KS, :]),
             wts.wa_shards()[c]) for c in range(NCORES)]


def _exact_actions(logits_bf, s_rows, wts):
    """Exact fp32 argmax via host recompute of the bf16 top-32 candidates."""
    n = s_rows.shape[0]
    ncand = min(32, NA)
    cand = np.argpartition(-logits_bf, ncand - 1, axis=1)[:, :ncand]
    cand.sort(axis=1)  # ties in exact fp32 break to the lowest column index
    actions = np.empty(n, np.int64)
    for i in range(n):
        exact = s_rows[i] @ wts.Wa[:, cand[i]]
        actions[i] = cand[i][np.argmax(exact)]
    return actions


def _w2_inputs(h, n_pad, wts):
    """Per-core GEMM inputs for the full-width z2 = h @ W2 (single-pass fp16)."""
    hT16 = _pad_rows_T(h, n_pad, np.float16)
    return [(hT16, wts.w2_shards()[c]) for c in range(NCORES)]


def _apply_bulk(state, pend, w2_outs):
    """Write a deferred device W2 product into state, then restore the
    exactly-recomputed decision columns and rows on top of it."""
    idx = pend["idx"]
    n = idx.size
    z2 = np.concatenate(w2_outs, axis=1)[:n]
    state[idx] = _sigmoid(z2)
    state[np.ix_(idx, COLS_EXACT)] = pend["cols"]
    if pend["exact"] is not None:
        need, vals = pend["exact"]
        state[need] = vals


# Ragged tails up to this many rows finalize their full-width W2 product
# exactly on host (<1% of the FLOPs) instead of paying a device launch.
HOST_TAIL_MAX = 128


def _step_reward(s):
    return np.float32(1000.0) * (
        s[:, IDX_EN_VLN] - s[:, IDX_MY_VLN]
        + (s[:, IDX_EN_DMG] - s[:, IDX_MY_DMG]))


def _players_winners(s):
    return np.argmax(s[:, 0:3], axis=1), np.argmax(s[:, 3:6], axis=1)


def kernel(initial_state, initial_action, W1, b1, W2, act_emb, Wa):
    global HW_NS
    HW_NS = 0.0
    state = np.array(initial_state, np.float32, copy=True)
    action0 = np.asarray(initial_action).astype(np.int64)
    wts = _Weights(W1, b1, W2, act_emb, Wa)

    player, winner = _players_winners(state)
    initial_player = player.copy()
    initial_done = (player == 0) | (winner > 0)
    reward = np.zeros(B, np.float32)

    # Deferred full-width W2 product of the previous step's rows.  Gates and
    # rewards never need it (the rescued columns are exact), and next-step GEMM
    # inputs are recomputed exactly on host, so the bulk product rides along in
    # the NEXT step's NEFF (or finalizes on host for a tiny tail).
    pending = None

    for i in range(T):
        player, winner = _players_winners(state)
        done = (player == 0) | (winner > 0)
        if i == 0:
            active = ~done
        else:
            active = (~done) & (player != initial_player)
        idx = np.where(active)[0]
        if idx.size == 0:
            break  # state is frozen for this and every later step

        if pending is not None:
            # Rows entering this step's GEMMs were updated last step; their
            # full-width state is recomputed exactly on host from exact h.
            need = np.intersect1d(idx, pending["idx"])
            if need.size:
                pos = np.searchsorted(pending["idx"], need)
                vals = _sigmoid(pending["h"][pos] @ wts.W2)
                state[need] = vals
                pending["exact"] = (need, vals)

        s_rows = state[idx]
        n = idx.size
        n_pad = _npad(n)
        sT = _pad_rows_T(s_rows, n_pad)

        if i == 0:
            act_rows = action0[idx]
            (z1_outs,) = _run_module(
                (_z1_spec(n_pad),), [_z1_inputs(sT, wts)])
        else:
            # One NEFF: this step's z1 (3-pass) + action logits (fp8) + the
            # previous step's deferred full-width W2 product.
            specs = [_z1_spec(n_pad), _logits_spec(n_pad)]
            ins = [_z1_inputs(sT, wts), _logits_inputs(sT.astype(FP8), wts)]
            if pending is not None:
                specs.append(_w2_spec(pending["n_pad"]))
                ins.append(_w2_inputs(pending["h"], pending["n_pad"], wts))
            outs = _run_module(tuple(specs), ins)
            z1_outs, lg_outs = outs[0], outs[1]
            if pending is not None:
                _apply_bulk(state, pending, outs[2])
                pending = None
            logits = np.zeros((n_pad, NA_PAD), np.float32)
            for o in lg_outs:
                logits += o
            act_rows = _exact_actions(logits[:n, :NA], s_rows, wts)

        z1 = z1_outs[0]
        for o in z1_outs[1:]:
            z1 = z1 + o
        h = np.tanh(z1[:n] + wts.act_emb[act_rows] + wts.b1).astype(np.float32)

        # exact decision + reward columns now; the bulk W2 product is deferred
        cols_sig = _sigmoid(h @ wts.W2[:, COLS_EXACT])
        state[np.ix_(idx, COLS_EXACT)] = cols_sig
        reward[idx] = (reward[idx] + _step_reward(state)[idx]).astype(np.float32)
        pending = {"idx": idx, "h": h, "cols": cols_sig, "n_pad": n_pad,
                   "exact": None}

    if pending is not None:
        if pending["idx"].size <= HOST_TAIL_MAX:
            # tiny ragged tail: finalize the last rows exactly on host
            state[pending["idx"]] = _sigmoid(pending["h"] @ wts.W2)
        else:
            (w2_outs,) = _run_module(
                (_w2_spec(pending["n_pad"]),),
                [_w2_inputs(pending["h"], pending["n_pad"], wts)])
            _apply_bulk(state, pending, w2_outs)

    player, winner = _players_winners(state)
    done = (player == 0) | (winner > 0)
    reward = (reward + np.where(~initial_done, np.float32(-0.01),
                                np.float32(0.0))).astype(np.float32)
    term = done & ~initial_done
    term_r = _step_reward(state) + np.float32(1000.0) * (
        state[:, IDX_EN_ACC] - state[:, IDX_MY_ACC])
    reward = (reward + np.where(term, term_r, np.float32(0.0))).astype(np.float32)
    return state, reward, done
